# revision 24
# baseline (speedup 1.0000x reference)
"""Trainium2 Bass kernel for nn_CrossAttention (B=4, SQ=1024, SKV=2048, D=1024, H=16).

Sharding: core c handles batch b = c//2 and head-group g = c%2 (8 heads each).
Per core (all matmuls in bf16 with f32 PSUM accumulation):
  qT  = qw_g^T @ x_b^T          [512d, 1024m]   (d-major so heads slice on partitions)
  kT  = kw_g^T @ enc_b^T        [512d, 2048n]
  V   = enc_b @ vw_g (+vb)      [2048n, 512d]   (+ a column of ones per head)
  per head pair (2j, 2j+1) — the two heads sit at partitions 0:64 / 64:128 of one
  d-chunk, so their K=64 score matmuls land in different PE row-groups and run
  concurrently when issued back-to-back:
    S^T[n,m] = kT_h^T.T @ qT_h  (scores transposed: partitions = skv)
    E^T      = exp(0.125*S^T + mask[n])          (ACT, bias = per-partition mask)
    [O^T | Z] = [V_h | 1].T @ E^T                (PV matmul; row 64 = softmax denom Z[m])
    rz[128,m] = broadcast(1/Z)  (reciprocal_approx_fast + K=1 ones matmul)
    P^T      = E^T * rz         -> attn_weights output (transposed; host views back)
    attnT_h  = O^T * rz[0:64]
  outp1/outp2 = attnT[heads 0:4 / 4:8].T @ pw_g  (host sums partials + bias)

Outputs per core: ptout [8, 2048, 1024] f32 (P^T per head), outp1/outp2 [1024, 1024] f32.
Host: attn_weights[b, g*8+h] = ptout[h].T (numpy view), attn_output = sum of partials + bias.
"""
import sys

if "/opt/trn_rl_repo" not in sys.path:
    sys.path.insert(0, "/opt/trn_rl_repo")

import numpy as np
import ml_dtypes

import concourse.bass as bass
import concourse.tile as tile
from concourse import bacc, mybir
from concourse.bass_utils import run_bass_kernel_spmd

BF16 = mybir.dt.bfloat16
F32 = mybir.dt.float32
AF = mybir.ActivationFunctionType

B, SQ, SKV, D, H = 4, 1024, 2048, 1024, 16
DL = D // 2          # 512 local d per core (8 heads)
KC = D // 128        # 8 contraction chunks
NCH = SKV // 128     # 16 skv chunks
MCH = SQ // 128      # 8 sq chunks
DCH = DL // 128      # 4 local-d chunks
HL = 8               # heads per core

_NC = None


def _build():
    nc = bacc.Bacc(None, target_bir_lowering=False)

    xT = nc.dram_tensor("xT", [D, SQ], BF16, kind="ExternalInput")
    encT = nc.dram_tensor("encT", [D, SKV], BF16, kind="ExternalInput")
    qw = nc.dram_tensor("qw", [D, DL], BF16, kind="ExternalInput")
    kw = nc.dram_tensor("kw", [D, DL], BF16, kind="ExternalInput")
    vw = nc.dram_tensor("vw", [D, DL], BF16, kind="ExternalInput")
    pw = nc.dram_tensor("pw", [DL, D], BF16, kind="ExternalInput")
    qb = nc.dram_tensor("qb", [128, DCH], F32, kind="ExternalInput")
    kb = nc.dram_tensor("kb", [128, DCH], F32, kind="ExternalInput")
    vb = nc.dram_tensor("vb", [1, DL], F32, kind="ExternalInput")
    mask = nc.dram_tensor("mask", [128, NCH], F32, kind="ExternalInput")

    ptout = nc.dram_tensor("ptout", [HL, SKV, SQ], F32, kind="ExternalOutput")
    outp1 = nc.dram_tensor("outp1", [SQ, D], F32, kind="ExternalOutput")
    outp2 = nc.dram_tensor("outp2", [SQ, D], F32, kind="ExternalOutput")

    with tile.TileContext(nc) as tc:
        import contextlib
        with contextlib.ExitStack() as ctx:
            persist = ctx.enter_context(tc.tile_pool(name="persist", bufs=1))
            poolA = ctx.enter_context(tc.tile_pool(name="poolA", bufs=1))
            poolB_cm = tc.tile_pool(name="poolB", bufs=1)
            poolB = poolB_cm.__enter__()
            psum = ctx.enter_context(tc.tile_pool(name="psum", bufs=2, space="PSUM"))

            # ---- constants / small inputs (needed early by ACT bias reads) ----
            ones_f = persist.tile([1, 128], F32, tag="ones_f")
            nc.vector.memset(ones_f, 1.0)
            qb_sb = persist.tile([128, DCH], F32, tag="qb")
            kb_sb = persist.tile([128, DCH], F32, tag="kb")
            vb_sb = persist.tile([1, DL], F32, tag="vb")
            mask_sb = persist.tile([128, NCH], F32, tag="mask")
            nc.sync.dma_start(out=kb_sb, in_=kb[:, :])
            nc.sync.dma_start(out=qb_sb, in_=qb[:, :])
            nc.sync.dma_start(out=vb_sb, in_=vb[:, :])
            nc.sync.dma_start(out=mask_sb, in_=mask[:, :])

            # ---- big inputs; per-K-chunk loads so matmuls start early ----
            kw_sb = poolA.tile([128, KC, DL], BF16, tag="kw")
            encT_sb = poolA.tile([128, KC, SKV], BF16, tag="encT")
            qw_sb = poolB.tile([128, KC, DL], BF16, tag="qw")
            xT_sb = poolB.tile([128, KC, SQ], BF16, tag="xT")
            vw_sb = poolA.tile([128, KC, DL], BF16, tag="vw")
            pw_sb = persist.tile([128, DCH, D], BF16, tag="pw")
            kw_r = kw[:, :].rearrange("(kc p) d -> p kc d", p=128)
            qw_r = qw[:, :].rearrange("(kc p) d -> p kc d", p=128)
            encT_r = encT[:, :].rearrange("(kc p) n -> p kc n", p=128)
            xT_r = xT[:, :].rearrange("(kc p) m -> p kc m", p=128)
            for kc in range(KC):
                nc.sync.dma_start(out=qw_sb[:, kc, :], in_=qw_r[:, kc, :])
                nc.sync.dma_start(out=xT_sb[:, kc, :], in_=xT_r[:, kc, :])
            for kc in range(KC):
                nc.sync.dma_start(out=kw_sb[:, kc, :], in_=kw_r[:, kc, :])
                nc.sync.dma_start(out=encT_sb[:, kc, :], in_=encT_r[:, kc, :])
            nc.sync.dma_start(out=vw_sb, in_=vw[:, :].rearrange("(kc p) d -> p kc d", p=128))
            nc.sync.dma_start(out=pw_sb, in_=pw[:, :].rearrange("(dc p) n -> p dc n", p=128))

            # ---- persistent intermediates ----
            qT_sb = persist.tile([128, DCH, SQ], BF16, tag="qT")
            kT_sb = persist.tile([128, DCH, SKV], BF16, tag="kT")
            V_sb = persist.tile([128, NCH, HL, 65], BF16, tag="V")
            attnT_sb = persist.tile([128, DCH, SQ], BF16, tag="attnT")
            nc.vector.memset(V_sb, 1.0)  # ones column (idx 64) survives the V copies

            def proj_qT(dc):
                ps = psum.tile([128, 1024], F32, tag="big")
                for mh in range(2):
                    for kc in range(KC):
                        nc.tensor.matmul(
                            ps[:, mh * 512:(mh + 1) * 512],
                            lhsT=qw_sb[:, kc, dc * 128:(dc + 1) * 128],
                            rhs=xT_sb[:, kc, mh * 512:(mh + 1) * 512],
                            start=(kc == 0), stop=(kc == KC - 1))
                nc.vector.tensor_scalar_add(
                    out=qT_sb[:, dc, :], in0=ps,
                    scalar1=qb_sb[:, dc:dc + 1])

            def proj_kT_nh(dc, nh):
                    ps = psum.tile([128, 1024], F32, tag="big",
                                   name=f"kps{dc}_{nh}")
                    for mh in range(2):
                        for kc in range(KC):
                            nc.tensor.matmul(
                                ps[:, mh * 512:(mh + 1) * 512],
                                lhsT=kw_sb[:, kc, dc * 128:(dc + 1) * 128],
                                rhs=encT_sb[:, kc, nh * 1024 + mh * 512:
                                            nh * 1024 + (mh + 1) * 512],
                                start=(kc == 0), stop=(kc == KC - 1))
                    nc.vector.tensor_scalar_add(
                        out=kT_sb[:, dc, nh * 1024:(nh + 1) * 1024], in0=ps,
                        scalar1=kb_sb[:, dc:dc + 1])

            def proj_V_chunk(ncn):
                ps = psum.tile([128, 1024], F32, tag="big", name=f"vps{ncn}")
                for kc in range(KC):
                    nc.tensor.matmul(
                        ps[:, 0:512],
                        lhsT=encT_sb[:, kc, ncn * 128:(ncn + 1) * 128],
                        rhs=vw_sb[:, kc, :],
                        start=(kc == 0), stop=False)
                nc.tensor.matmul(ps[:, 0:512], lhsT=ones_f, rhs=vb_sb,
                                 start=False, stop=True)
                nc.vector.tensor_copy(
                    out=V_sb[:, ncn, :, 0:64],
                    in_=ps[:, 0:512].rearrange("p (h d) -> p h d", h=HL))

            def head_pair(h0, work, etp, outs, emit_mid=None, emit_chunk=None):
                dc = h0 // 2
                pvs = [psum.tile([65, 1024], F32, tag="pv", name=f"pv{h0}_{i}") for i in range(2)]
                etss = [[], []]
                def emit_pv(ncn):
                    for i in range(2):
                        for mh in range(2):
                            nc.tensor.matmul(
                                pvs[i][:, mh * 512:(mh + 1) * 512],
                                lhsT=V_sb[:, ncn, h0 + i, :],
                                rhs=etss[i][ncn][:, mh * 512:(mh + 1) * 512],
                                start=(ncn == 0), stop=(ncn == NCH - 1))

                PV_LAG = 2  # PV consumes exps a couple of chunks old: no stall
                for ncn in range(NCH):
                    if emit_chunk is not None:
                        emit_chunk(ncn)
                    scs = [psum.tile([128, 1024], F32, tag="big", name=f"sc{h0}_{ncn}_{i}") for i in range(2)]
                    # interleave the two heads' K=64 matmuls so the PE runs
                    # them concurrently in different row-groups
                    for mh in range(2):
                        for i, hb in enumerate((0, 64)):
                            nc.tensor.matmul(
                                scs[i][:, mh * 512:(mh + 1) * 512],
                                lhsT=kT_sb[hb:hb + 64, dc,
                                           ncn * 128:(ncn + 1) * 128],
                                rhs=qT_sb[hb:hb + 64, dc,
                                          mh * 512:(mh + 1) * 512],
                                start=True, stop=True)
                    for i in range(2):
                        et = etp.tile([128, 1024], BF16, tag="et")
                        etss[i].append(et)
                        nc.scalar.activation(out=et, in_=scs[i], func=AF.Exp,
                                             bias=mask_sb[:, ncn:ncn + 1],
                                             scale=0.125)
                    if ncn >= PV_LAG:
                        emit_pv(ncn - PV_LAG)
                for ncn in range(NCH - PV_LAG, NCH):
                    emit_pv(ncn)
                rzs = []
                for i, hb in enumerate((0, 64)):
                    pv = pvs[i]
                    # 1/Z broadcast to all partitions via K=1 ones matmul
                    zrow = work.tile([1, 1024], F32, tag="zrow", bufs=1,
                                     name=f"zrow{h0}_{i}")
                    nc.scalar.copy(out=zrow, in_=pv[64:65, :])
                    rzrow = work.tile([1, 1024], F32, tag="rzrow", bufs=1,
                                      name=f"rzrow{h0}_{i}")
                    nc.vector.reciprocal_approx_fast(out=rzrow, in_=zrow)
                    bc = psum.tile([128, 1024], F32, tag="big",
                                   name=f"bc{h0}_{i}")
                    for mh in range(2):
                        nc.tensor.matmul(bc[:, mh * 512:(mh + 1) * 512],
                                         lhsT=ones_f,
                                         rhs=rzrow[:, mh * 512:(mh + 1) * 512],
                                         start=True, stop=True)
                    rz = work.tile([128, 1024], BF16, tag="rz",
                                   name=f"rz{h0}_{i}")
                    nc.scalar.activation(out=rz, in_=bc, func=AF.Copy)
                    rzs.append(rz)
                    # critical path: c_proj + the next pair gate on attnT
                    nc.vector.tensor_mul(out=attnT_sb[hb:hb + 64, dc, :],
                                         in0=pv[0:64, :], in1=rz[0:64, :])
                if emit_mid is not None:
                    emit_mid()
                # bulk: normalized probabilities in bf16 (DVE 2x mode); the
                # SWDGE DMA upcasts bf16->f32 on the way to DRAM
                for ncn in range(NCH):
                    for i in range(2):
                        pt = outs.tile([128, 1024], BF16, tag="pt",
                                       name=f"pt{h0}_{i}_{ncn}")
                        if ncn in (4, 9, 14):
                            nc.gpsimd.tensor_mul(out=pt, in0=etss[i][ncn],
                                                 in1=rzs[i])
                        else:
                            nc.vector.tensor_mul(out=pt, in0=etss[i][ncn],
                                                 in1=rzs[i])
                        nc.gpsimd.dma_start(
                            out=ptout[h0 + i, ncn * 128:(ncn + 1) * 128, :],
                            in_=pt)

            def c_proj(outp, dcs, outs):
                for mc in range(MCH):
                    ps = psum.tile([128, 1024], F32, tag="pv", name=f"cps{mc}")
                    for mh in range(2):
                        for j, dc in enumerate(dcs):
                            nc.tensor.matmul(
                                ps[:, mh * 512:(mh + 1) * 512],
                                lhsT=attnT_sb[:, dc, mc * 128:(mc + 1) * 128],
                                rhs=pw_sb[:, dc, mh * 512:(mh + 1) * 512],
                                start=(j == 0), stop=(j == len(dcs) - 1))
                    ost = outs.tile([128, 1024], F32, tag="ost")
                    nc.vector.tensor_copy(out=ost, in_=ps)
                    nc.sync.dma_start(out=outp[mc * 128:(mc + 1) * 128, :], in_=ost)

            # phase 1: qT for all d-chunks (frees poolB) + kT dc0
            proj_qT(0)
            proj_kT_nh(0, 0)
            proj_kT_nh(0, 1)
            for dc in range(1, DCH):
                proj_qT(dc)
            poolB_cm.__exit__(None, None, None)

            # order matters: work+outs tiles are first touched late (pair
            # tails), so THEY absorb the address-reuse dependency on poolB;
            # the et pool lands above poolB's range and exp can start early
            work = ctx.enter_context(tc.tile_pool(name="work", bufs=2))
            outs = ctx.enter_context(tc.tile_pool(name="outs", bufs=3))
            etp = ctx.enter_context(tc.tile_pool(name="et", bufs=33))

            def chunk_work(pair_idx):
                # interleave V (pair 0) and the next pair's kT bursts into
                # this pair's chunk loop so the PE never starves
                def emit(ncn):
                    if pair_idx == 0:
                        proj_V_chunk(ncn)
                    if pair_idx < DCH - 1:
                        if ncn == 8:
                            proj_kT_nh(pair_idx + 1, 0)
                        elif ncn == 12:
                            proj_kT_nh(pair_idx + 1, 1)
                return emit

            head_pair(0, work, etp, outs, emit_chunk=chunk_work(0))
            head_pair(2, work, etp, outs, emit_chunk=chunk_work(1),
                      emit_mid=lambda: c_proj(outp1, (0, 1), outs))
            head_pair(4, work, etp, outs, emit_chunk=chunk_work(2))
            head_pair(6, work, etp, outs, emit_chunk=chunk_work(3),
                      emit_mid=lambda: c_proj(outp2, (2, 3), outs))

    nc.compile()
    return nc


def _get_nc():
    global _NC
    if _NC is None:
        _NC = _build()
    return _NC


def _shard(inputs):
    bf16 = ml_dtypes.bfloat16
    x = np.asarray(inputs["hidden_states"], np.float32)
    enc = np.asarray(inputs["encoder_hidden_states"], np.float32)
    maskf = np.asarray(inputs["attention_mask_kv"], np.float32)
    qw = np.asarray(inputs["q_attn_w"], np.float32)
    qb = np.asarray(inputs["q_attn_b"], np.float32)
    cw = np.asarray(inputs["c_attn_w"], np.float32)
    cb = np.asarray(inputs["c_attn_b"], np.float32)
    pwf = np.asarray(inputs["c_proj_w"], np.float32)

    in_maps = []
    for c in range(8):
        b, g = divmod(c, 2)
        sl = slice(g * DL, (g + 1) * DL)
        vsl = slice(D + g * DL, D + (g + 1) * DL)
        in_maps.append({
            "xT": np.ascontiguousarray(x[b].T).astype(bf16),
            "encT": np.ascontiguousarray(enc[b].T).astype(bf16),
            "qw": np.ascontiguousarray(qw[:, sl]).astype(bf16),
            "kw": np.ascontiguousarray(cw[:, sl]).astype(bf16),
            "vw": np.ascontiguousarray(cw[:, vsl]).astype(bf16),
            "pw": np.ascontiguousarray(pwf[sl, :]).astype(bf16),
            "qb": np.ascontiguousarray(qb[sl].reshape(DCH, 128).T),
            "kb": np.ascontiguousarray(cb[sl].reshape(DCH, 128).T),
            "vb": np.ascontiguousarray(cb[vsl].reshape(1, DL)),
            "mask": np.ascontiguousarray(maskf[b, 0, 0].reshape(NCH, 128).T),
        })
    return in_maps


def kernel(**inputs):
    nc = _get_nc()
    in_maps = _shard(inputs)
    res = run_bass_kernel_spmd(nc, in_maps, core_ids=list(range(8)))

    pb = np.asarray(inputs["c_proj_b"], np.float32)
    attn_w = np.empty((B, H, SQ, SKV), np.float32)
    out = np.zeros((B, SQ, D), np.float32)
    for c in range(8):
        b, g = divmod(c, 2)
        pt = res.results[c]["ptout"]           # [8, SKV, SQ]
        attn_w[b, g * HL:(g + 1) * HL] = pt.transpose(0, 2, 1)
        out[b] += res.results[c]["outp1"]
        out[b] += res.results[c]["outp2"]
    out += pb
    return out, attn_w


# revision 25
# speedup vs baseline: 1.1282x; 1.1282x over previous
"""Trainium2 Bass kernel for nn_CrossAttention (B=4, SQ=1024, SKV=2048, D=1024, H=16).

Sharding: core c handles batch b = c//2 and head-group g = c%2 (8 heads each).
Per core (all matmuls in bf16 with f32 PSUM accumulation):
  qT  = qw_g^T @ x_b^T          [512d, 1024m]   (d-major so heads slice on partitions)
  kT  = kw_g^T @ enc_b^T        [512d, 2048n]
  V   = enc_b @ vw_g (+vb)      [2048n, 512d]   (+ a column of ones per head)
  per head pair (2j, 2j+1) — the two heads sit at partitions 0:64 / 64:128 of one
  d-chunk, so their K=64 score matmuls land in different PE row-groups and run
  concurrently when issued back-to-back:
    S^T[n,m] = kT_h^T.T @ qT_h  (scores transposed: partitions = skv)
    E^T      = exp(0.125*S^T + mask[n])          (ACT, bias = per-partition mask)
    [O^T | Z] = [V_h | 1].T @ E^T                (PV matmul; row 64 = softmax denom Z[m])
    rz[128,m] = broadcast(1/Z)  (reciprocal_approx_fast + K=1 ones matmul)
    P^T      = E^T * rz         -> attn_weights output (transposed; host views back)
    attnT_h  = O^T * rz[0:64]
  outp1/outp2 = attnT[heads 0:4 / 4:8].T @ pw_g  (host sums partials + bias)

Outputs per core: ptout [8, 2048, 1024] f32 (P^T per head), outp1/outp2 [1024, 1024] f32.
Host: attn_weights[b, g*8+h] = ptout[h].T (numpy view), attn_output = sum of partials + bias.
"""
import sys

if "/opt/trn_rl_repo" not in sys.path:
    sys.path.insert(0, "/opt/trn_rl_repo")

import numpy as np
import ml_dtypes

import concourse.bass as bass
import concourse.tile as tile
from concourse import bacc, mybir
from concourse.bass_utils import run_bass_kernel_spmd

BF16 = mybir.dt.bfloat16
F32 = mybir.dt.float32
AF = mybir.ActivationFunctionType

B, SQ, SKV, D, H = 4, 1024, 2048, 1024, 16
DL = D // 2          # 512 local d per core (8 heads)
KC = D // 128        # 8 contraction chunks
NCH = SKV // 128     # 16 skv chunks
MCH = SQ // 128      # 8 sq chunks
DCH = DL // 128      # 4 local-d chunks
HL = 8               # heads per core

_NC = None


def _build():
    nc = bacc.Bacc(None, target_bir_lowering=False)

    xT = nc.dram_tensor("xT", [D, SQ], BF16, kind="ExternalInput")
    encT = nc.dram_tensor("encT", [D, SKV], BF16, kind="ExternalInput")
    qw = nc.dram_tensor("qw", [D, DL], BF16, kind="ExternalInput")
    kw = nc.dram_tensor("kw", [D, DL], BF16, kind="ExternalInput")
    vw = nc.dram_tensor("vw", [D, DL], BF16, kind="ExternalInput")
    pw = nc.dram_tensor("pw", [DL, D], BF16, kind="ExternalInput")
    qb = nc.dram_tensor("qb", [128, DCH], F32, kind="ExternalInput")
    kb = nc.dram_tensor("kb", [128, DCH], F32, kind="ExternalInput")
    mask = nc.dram_tensor("mask", [128, NCH], F32, kind="ExternalInput")

    ptout = nc.dram_tensor("ptout", [HL, SKV, SQ], F32, kind="ExternalOutput")
    outp1 = nc.dram_tensor("outp1", [SQ, D], F32, kind="ExternalOutput")
    outp2 = nc.dram_tensor("outp2", [SQ, D], F32, kind="ExternalOutput")

    with tile.TileContext(nc) as tc:
        import contextlib
        with contextlib.ExitStack() as ctx:
            persist = ctx.enter_context(tc.tile_pool(name="persist", bufs=1))
            poolA = ctx.enter_context(tc.tile_pool(name="poolA", bufs=1))
            poolB_cm = tc.tile_pool(name="poolB", bufs=1)
            poolB = poolB_cm.__enter__()
            psum = ctx.enter_context(tc.tile_pool(name="psum", bufs=2, space="PSUM"))

            # ---- constants / small inputs (needed early by ACT bias reads) ----
            ones_bf = persist.tile([1, 128], BF16, tag="ones_bf")
            nc.vector.memset(ones_bf, 1.0)
            qb_sb = persist.tile([128, DCH], F32, tag="qb")
            kb_sb = persist.tile([128, DCH], F32, tag="kb")
            mask_sb = persist.tile([128, NCH], F32, tag="mask")
            nc.sync.dma_start(out=kb_sb, in_=kb[:, :])
            nc.sync.dma_start(out=qb_sb, in_=qb[:, :])
            nc.sync.dma_start(out=mask_sb, in_=mask[:, :])

            # ---- big inputs; per-K-chunk loads so matmuls start early ----
            kw_sb = poolA.tile([128, KC, DL], BF16, tag="kw")
            encT_sb = poolA.tile([128, KC, SKV], BF16, tag="encT")
            qw_sb = poolB.tile([128, KC, DL], BF16, tag="qw")
            xT_sb = poolB.tile([128, KC, SQ], BF16, tag="xT")
            vw_sb = poolA.tile([128, KC, DL], BF16, tag="vw")
            pw_sb = persist.tile([128, DCH, D], BF16, tag="pw")
            kw_r = kw[:, :].rearrange("(kc p) d -> p kc d", p=128)
            qw_r = qw[:, :].rearrange("(kc p) d -> p kc d", p=128)
            encT_r = encT[:, :].rearrange("(kc p) n -> p kc n", p=128)
            xT_r = xT[:, :].rearrange("(kc p) m -> p kc m", p=128)
            for kc in range(KC):
                nc.sync.dma_start(out=qw_sb[:, kc, :], in_=qw_r[:, kc, :])
                nc.sync.dma_start(out=xT_sb[:, kc, :], in_=xT_r[:, kc, :])
            for kc in range(KC):
                nc.sync.dma_start(out=kw_sb[:, kc, :], in_=kw_r[:, kc, :])
                nc.sync.dma_start(out=encT_sb[:, kc, :], in_=encT_r[:, kc, :])
            nc.sync.dma_start(out=vw_sb, in_=vw[:, :].rearrange("(kc p) d -> p kc d", p=128))
            nc.sync.dma_start(out=pw_sb, in_=pw[:, :].rearrange("(dc p) n -> p dc n", p=128))

            # ---- persistent intermediates ----
            qT_sb = persist.tile([128, DCH, SQ], BF16, tag="qT")
            kT_sb = persist.tile([128, DCH, SKV], BF16, tag="kT")
            V_sb = persist.tile([128, NCH, HL, 65], BF16, tag="V")
            attnT_sb = persist.tile([128, DCH, SQ], BF16, tag="attnT")
            nc.vector.memset(V_sb, 1.0)  # ones column (idx 64) survives the V copies

            def proj_qT(dc):
                ps = psum.tile([128, 1024], F32, tag="big")
                for mh in range(2):
                    for kc in range(KC):
                        nc.tensor.matmul(
                            ps[:, mh * 512:(mh + 1) * 512],
                            lhsT=qw_sb[:, kc, dc * 128:(dc + 1) * 128],
                            rhs=xT_sb[:, kc, mh * 512:(mh + 1) * 512],
                            start=(kc == 0), stop=(kc == KC - 1))
                nc.vector.tensor_scalar_add(
                    out=qT_sb[:, dc, :], in0=ps,
                    scalar1=qb_sb[:, dc:dc + 1])

            def proj_kT_nh(dc, nh):
                    ps = psum.tile([128, 1024], F32, tag="big",
                                   name=f"kps{dc}_{nh}")
                    for mh in range(2):
                        for kc in range(KC):
                            nc.tensor.matmul(
                                ps[:, mh * 512:(mh + 1) * 512],
                                lhsT=kw_sb[:, kc, dc * 128:(dc + 1) * 128],
                                rhs=encT_sb[:, kc, nh * 1024 + mh * 512:
                                            nh * 1024 + (mh + 1) * 512],
                                start=(kc == 0), stop=(kc == KC - 1))
                    nc.vector.tensor_scalar_add(
                        out=kT_sb[:, dc, nh * 1024:(nh + 1) * 1024], in0=ps,
                        scalar1=kb_sb[:, dc:dc + 1])

            def proj_V_chunk(ncn):
                ps = psum.tile([128, 1024], F32, tag="big", name=f"vps{ncn}")
                for kc in range(KC):
                    nc.tensor.matmul(
                        ps[:, 0:512],
                        lhsT=encT_sb[:, kc, ncn * 128:(ncn + 1) * 128],
                        rhs=vw_sb[:, kc, :],
                        start=(kc == 0), stop=False)
                nc.vector.tensor_copy(
                    out=V_sb[:, ncn, :, 0:64],
                    in_=ps[:, 0:512].rearrange("p (h d) -> p h d", h=HL))

            def head_pair(h0, work, etp, outs, emit_mid=None, emit_chunk=None):
                dc = h0 // 2
                pvs = [psum.tile([65, 1024], F32, tag="pv", name=f"pv{h0}_{i}") for i in range(2)]
                etss = [[], []]
                def emit_pv(ncn):
                    for i in range(2):
                        for mh in range(2):
                            nc.tensor.matmul(
                                pvs[i][:, mh * 512:(mh + 1) * 512],
                                lhsT=V_sb[:, ncn, h0 + i, :],
                                rhs=etss[i][ncn][:, mh * 512:(mh + 1) * 512],
                                start=(ncn == 0), stop=(ncn == NCH - 1))

                PV_LAG = 2  # PV consumes exps a couple of chunks old: no stall
                for ncn in range(NCH):
                    if emit_chunk is not None:
                        emit_chunk(ncn)
                    scs = [psum.tile([128, 1024], F32, tag="big", name=f"sc{h0}_{ncn}_{i}") for i in range(2)]
                    # interleave the two heads' K=64 matmuls so the PE runs
                    # them concurrently in different row-groups
                    for mh in range(2):
                        for i, hb in enumerate((0, 64)):
                            nc.tensor.matmul(
                                scs[i][:, mh * 512:(mh + 1) * 512],
                                lhsT=kT_sb[hb:hb + 64, dc,
                                           ncn * 128:(ncn + 1) * 128],
                                rhs=qT_sb[hb:hb + 64, dc,
                                          mh * 512:(mh + 1) * 512],
                                start=True, stop=True)
                    for i in range(2):
                        et = etp.tile([128, 1024], BF16, tag="et")
                        etss[i].append(et)
                        nc.scalar.activation(out=et, in_=scs[i], func=AF.Exp,
                                             bias=mask_sb[:, ncn:ncn + 1],
                                             scale=0.125)
                    if ncn >= PV_LAG:
                        emit_pv(ncn - PV_LAG)
                for ncn in range(NCH - PV_LAG, NCH):
                    emit_pv(ncn)
                rzs = []
                for i, hb in enumerate((0, 64)):
                    pv = pvs[i]
                    # 1/Z broadcast to all partitions via K=1 ones matmul
                    zrow = work.tile([1, 1024], F32, tag="zrow", bufs=1,
                                     name=f"zrow{h0}_{i}")
                    nc.scalar.copy(out=zrow, in_=pv[64:65, :])
                    rzrow = work.tile([1, 1024], F32, tag="rzrow", bufs=1,
                                      name=f"rzrow{h0}_{i}")
                    nc.vector.reciprocal_approx_fast(out=rzrow, in_=zrow)
                    rzrow_bf = work.tile([1, 1024], BF16, tag="rzrow_bf",
                                         bufs=1, name=f"rzbf{h0}_{i}")
                    nc.vector.tensor_copy(out=rzrow_bf, in_=rzrow)
                    bc = psum.tile([128, 1024], F32, tag="big",
                                   name=f"bc{h0}_{i}")
                    for mh in range(2):
                        nc.tensor.matmul(bc[:, mh * 512:(mh + 1) * 512],
                                         lhsT=ones_bf,
                                         rhs=rzrow_bf[:, mh * 512:(mh + 1) * 512],
                                         start=True, stop=True)
                    rz = work.tile([128, 1024], BF16, tag="rz",
                                   name=f"rz{h0}_{i}")
                    nc.scalar.activation(out=rz, in_=bc, func=AF.Copy)
                    rzs.append(rz)
                    # critical path: c_proj + the next pair gate on attnT
                    nc.vector.tensor_mul(out=attnT_sb[hb:hb + 64, dc, :],
                                         in0=pv[0:64, :], in1=rz[0:64, :])
                if emit_mid is not None:
                    emit_mid()
                # bulk: normalized probabilities in bf16 (DVE 2x mode); the
                # SWDGE DMA upcasts bf16->f32 on the way to DRAM
                for ncn in range(NCH):
                    for i in range(2):
                        pt = outs.tile([128, 1024], BF16, tag="pt",
                                       name=f"pt{h0}_{i}_{ncn}")
                        if ncn in (4, 9, 14):
                            nc.gpsimd.tensor_mul(out=pt, in0=etss[i][ncn],
                                                 in1=rzs[i])
                        else:
                            nc.vector.tensor_mul(out=pt, in0=etss[i][ncn],
                                                 in1=rzs[i])
                        nc.gpsimd.dma_start(
                            out=ptout[h0 + i, ncn * 128:(ncn + 1) * 128, :],
                            in_=pt)

            def c_proj(outp, dcs, outs):
                for mc in range(MCH):
                    ps = psum.tile([128, 1024], F32, tag="pv", name=f"cps{mc}")
                    for mh in range(2):
                        for j, dc in enumerate(dcs):
                            nc.tensor.matmul(
                                ps[:, mh * 512:(mh + 1) * 512],
                                lhsT=attnT_sb[:, dc, mc * 128:(mc + 1) * 128],
                                rhs=pw_sb[:, dc, mh * 512:(mh + 1) * 512],
                                start=(j == 0), stop=(j == len(dcs) - 1))
                    ost = outs.tile([128, 1024], F32, tag="ost")
                    nc.vector.tensor_copy(out=ost, in_=ps)
                    nc.sync.dma_start(out=outp[mc * 128:(mc + 1) * 128, :], in_=ost)

            # phase 1: qT for all d-chunks (frees poolB) + kT dc0
            proj_qT(0)
            proj_kT_nh(0, 0)
            proj_kT_nh(0, 1)
            for dc in range(1, DCH):
                proj_qT(dc)
            poolB_cm.__exit__(None, None, None)

            # order matters: work+outs tiles are first touched late (pair
            # tails), so THEY absorb the address-reuse dependency on poolB;
            # the et pool lands above poolB's range and exp can start early
            work = ctx.enter_context(tc.tile_pool(name="work", bufs=2))
            outs = ctx.enter_context(tc.tile_pool(name="outs", bufs=3))
            etp = ctx.enter_context(tc.tile_pool(name="et", bufs=34))

            def chunk_work(pair_idx):
                # interleave V (pair 0) and the next pair's kT bursts into
                # this pair's chunk loop so the PE never starves
                def emit(ncn):
                    if pair_idx == 0:
                        proj_V_chunk(ncn)
                    if pair_idx < DCH - 1:
                        if ncn == 8:
                            proj_kT_nh(pair_idx + 1, 0)
                        elif ncn == 12:
                            proj_kT_nh(pair_idx + 1, 1)
                return emit

            head_pair(0, work, etp, outs, emit_chunk=chunk_work(0))
            head_pair(2, work, etp, outs, emit_chunk=chunk_work(1),
                      emit_mid=lambda: c_proj(outp1, (0, 1), outs))
            head_pair(4, work, etp, outs, emit_chunk=chunk_work(2))
            head_pair(6, work, etp, outs, emit_chunk=chunk_work(3),
                      emit_mid=lambda: c_proj(outp2, (2, 3), outs))

    nc.compile()
    return nc


def _get_nc():
    global _NC
    if _NC is None:
        _NC = _build()
    return _NC


def _shard(inputs):
    bf16 = ml_dtypes.bfloat16
    x = np.asarray(inputs["hidden_states"], np.float32)
    enc = np.asarray(inputs["encoder_hidden_states"], np.float32)
    maskf = np.asarray(inputs["attention_mask_kv"], np.float32)
    qw = np.asarray(inputs["q_attn_w"], np.float32)
    qb = np.asarray(inputs["q_attn_b"], np.float32)
    cw = np.asarray(inputs["c_attn_w"], np.float32)
    cb = np.asarray(inputs["c_attn_b"], np.float32)
    pwf = np.asarray(inputs["c_proj_w"], np.float32)

    in_maps = []
    for c in range(8):
        b, g = divmod(c, 2)
        sl = slice(g * DL, (g + 1) * DL)
        vsl = slice(D + g * DL, D + (g + 1) * DL)
        in_maps.append({
            "xT": np.ascontiguousarray(x[b].T).astype(bf16),
            "encT": np.ascontiguousarray(enc[b].T).astype(bf16),
            "qw": np.ascontiguousarray(qw[:, sl]).astype(bf16),
            "kw": np.ascontiguousarray(cw[:, sl]).astype(bf16),
            "vw": np.ascontiguousarray(cw[:, vsl]).astype(bf16),
            "pw": np.ascontiguousarray(pwf[sl, :]).astype(bf16),
            "qb": np.ascontiguousarray(qb[sl].reshape(DCH, 128).T),
            "kb": np.ascontiguousarray(cb[sl].reshape(DCH, 128).T),
            "mask": np.ascontiguousarray(maskf[b, 0, 0].reshape(NCH, 128).T),
        })
    return in_maps


def kernel(**inputs):
    nc = _get_nc()
    in_maps = _shard(inputs)
    res = run_bass_kernel_spmd(nc, in_maps, core_ids=list(range(8)))

    pb = np.asarray(inputs["c_proj_b"], np.float32)
    cb = np.asarray(inputs["c_attn_b"], np.float32)
    pwf = np.asarray(inputs["c_proj_w"], np.float32)
    attn_w = np.empty((B, H, SQ, SKV), np.float32)
    out = np.zeros((B, SQ, D), np.float32)
    for c in range(8):
        b, g = divmod(c, 2)
        pt = res.results[c]["ptout"]           # [8, SKV, SQ]
        attn_w[b, g * HL:(g + 1) * HL] = pt.transpose(0, 2, 1)
        out[b] += res.results[c]["outp1"]
        out[b] += res.results[c]["outp2"]
        out[b] += cb[D + g * DL:D + (g + 1) * DL] @ pwf[g * DL:(g + 1) * DL, :]
    out += pb
    return out, attn_w


# revision 27
# speedup vs baseline: 1.1314x; 1.0029x over previous
"""Trainium2 Bass kernel for nn_CrossAttention (B=4, SQ=1024, SKV=2048, D=1024, H=16).

Sharding: core c handles batch b = c//2 and head-group g = c%2 (8 heads each).
Per core (all matmuls in bf16 with f32 PSUM accumulation):
  qT  = qw_g^T @ x_b^T          [512d, 1024m]   (d-major so heads slice on partitions)
  kT  = kw_g^T @ enc_b^T        [512d, 2048n]
  V   = enc_b @ vw_g (+vb)      [2048n, 512d]   (+ a column of ones per head)
  per head pair (2j, 2j+1) — the two heads sit at partitions 0:64 / 64:128 of one
  d-chunk, so their K=64 score matmuls land in different PE row-groups and run
  concurrently when issued back-to-back:
    S^T[n,m] = kT_h^T.T @ qT_h  (scores transposed: partitions = skv)
    E^T      = exp(0.125*S^T + mask[n])          (ACT, bias = per-partition mask)
    [O^T | Z] = [V_h | 1].T @ E^T                (PV matmul; row 64 = softmax denom Z[m])
    rz[128,m] = broadcast(1/Z)  (reciprocal_approx_fast + K=1 ones matmul)
    P^T      = E^T * rz         -> attn_weights output (transposed; host views back)
    attnT_h  = O^T * rz[0:64]
  outp1/outp2 = attnT[heads 0:4 / 4:8].T @ pw_g  (host sums partials + bias)

Outputs per core: ptout [8, 2048, 1024] f32 (P^T per head), outp1/outp2 [1024, 1024] f32.
Host: attn_weights[b, g*8+h] = ptout[h].T (numpy view), attn_output = sum of partials + bias.
"""
import sys

if "/opt/trn_rl_repo" not in sys.path:
    sys.path.insert(0, "/opt/trn_rl_repo")

import numpy as np
import ml_dtypes

import concourse.bass as bass
import concourse.tile as tile
from concourse import bacc, mybir
from concourse.bass_utils import run_bass_kernel_spmd

BF16 = mybir.dt.bfloat16
F32 = mybir.dt.float32
AF = mybir.ActivationFunctionType

B, SQ, SKV, D, H = 4, 1024, 2048, 1024, 16
DL = D // 2          # 512 local d per core (8 heads)
KC = D // 128        # 8 contraction chunks
NCH = SKV // 128     # 16 skv chunks
MCH = SQ // 128      # 8 sq chunks
DCH = DL // 128      # 4 local-d chunks
HL = 8               # heads per core

_NC = None


def _build():
    nc = bacc.Bacc(None, target_bir_lowering=False)

    xT = nc.dram_tensor("xT", [D, SQ], BF16, kind="ExternalInput")
    encT = nc.dram_tensor("encT", [D, SKV], BF16, kind="ExternalInput")
    qw = nc.dram_tensor("qw", [D, DL], BF16, kind="ExternalInput")
    kw = nc.dram_tensor("kw", [D, DL], BF16, kind="ExternalInput")
    vw = nc.dram_tensor("vw", [D, DL], BF16, kind="ExternalInput")
    pw = nc.dram_tensor("pw", [DL, D], BF16, kind="ExternalInput")
    qb = nc.dram_tensor("qb", [128, DCH], F32, kind="ExternalInput")
    kb = nc.dram_tensor("kb", [128, DCH], F32, kind="ExternalInput")
    mask = nc.dram_tensor("mask", [128, NCH], F32, kind="ExternalInput")

    ptout = nc.dram_tensor("ptout", [HL, SKV, SQ], F32, kind="ExternalOutput")
    outp1 = nc.dram_tensor("outp1", [SQ, D], F32, kind="ExternalOutput")
    outp2 = nc.dram_tensor("outp2", [SQ, D], F32, kind="ExternalOutput")

    with tile.TileContext(nc) as tc:
        import contextlib
        with contextlib.ExitStack() as ctx:
            persist = ctx.enter_context(tc.tile_pool(name="persist", bufs=1))
            poolA = ctx.enter_context(tc.tile_pool(name="poolA", bufs=1))
            poolB_cm = tc.tile_pool(name="poolB", bufs=1)
            poolB = poolB_cm.__enter__()
            psum = ctx.enter_context(tc.tile_pool(name="psum", bufs=2, space="PSUM"))

            # ---- constants / small inputs (needed early by ACT bias reads) ----
            ones_bf = persist.tile([1, 128], BF16, tag="ones_bf")
            nc.vector.memset(ones_bf, 1.0)
            qb_sb = persist.tile([128, DCH], F32, tag="qb")
            kb_sb = persist.tile([128, DCH], F32, tag="kb")
            mask_sb = persist.tile([128, NCH], F32, tag="mask")
            nc.sync.dma_start(out=kb_sb, in_=kb[:, :])
            nc.sync.dma_start(out=qb_sb, in_=qb[:, :])
            nc.sync.dma_start(out=mask_sb, in_=mask[:, :])

            # ---- big inputs; per-K-chunk loads so matmuls start early ----
            kw_sb = poolA.tile([128, KC, DL], BF16, tag="kw")
            encT_sb = poolA.tile([128, KC, SKV], BF16, tag="encT")
            qw_sb = poolB.tile([128, KC, DL], BF16, tag="qw")
            xT_sb = poolB.tile([128, KC, SQ], BF16, tag="xT")
            vw_sb = poolA.tile([128, KC, DL], BF16, tag="vw")
            pw_sb = persist.tile([128, DCH, D], BF16, tag="pw")
            kw_r = kw[:, :].rearrange("(kc p) d -> p kc d", p=128)
            qw_r = qw[:, :].rearrange("(kc p) d -> p kc d", p=128)
            encT_r = encT[:, :].rearrange("(kc p) n -> p kc n", p=128)
            xT_r = xT[:, :].rearrange("(kc p) m -> p kc m", p=128)
            for kc in range(KC):
                nc.sync.dma_start(out=qw_sb[:, kc, :], in_=qw_r[:, kc, :])
                nc.sync.dma_start(out=xT_sb[:, kc, :], in_=xT_r[:, kc, :])
            for kc in range(KC):
                nc.sync.dma_start(out=kw_sb[:, kc, :], in_=kw_r[:, kc, :])
                nc.sync.dma_start(out=encT_sb[:, kc, :], in_=encT_r[:, kc, :])
            nc.sync.dma_start(out=vw_sb, in_=vw[:, :].rearrange("(kc p) d -> p kc d", p=128))
            nc.sync.dma_start(out=pw_sb, in_=pw[:, :].rearrange("(dc p) n -> p dc n", p=128))

            # ---- persistent intermediates ----
            qT_sb = persist.tile([128, DCH, SQ], BF16, tag="qT")
            kT_sb = persist.tile([128, DCH, SKV], BF16, tag="kT")
            V_sb = persist.tile([128, NCH, HL, 65], BF16, tag="V")
            attnT_sb = persist.tile([128, DCH, SQ], BF16, tag="attnT")
            nc.vector.memset(V_sb, 1.0)  # ones column (idx 64) survives the V copies

            def proj_qT(dc):
                ps = psum.tile([128, 1024], F32, tag="big")
                for mh in range(2):
                    for kc in range(KC):
                        nc.tensor.matmul(
                            ps[:, mh * 512:(mh + 1) * 512],
                            lhsT=qw_sb[:, kc, dc * 128:(dc + 1) * 128],
                            rhs=xT_sb[:, kc, mh * 512:(mh + 1) * 512],
                            start=(kc == 0), stop=(kc == KC - 1))
                nc.vector.tensor_scalar_add(
                    out=qT_sb[:, dc, :], in0=ps,
                    scalar1=qb_sb[:, dc:dc + 1])

            def proj_kT_nh(dc, nh):
                    ps = psum.tile([128, 1024], F32, tag="big",
                                   name=f"kps{dc}_{nh}")
                    for mh in range(2):
                        for kc in range(KC):
                            nc.tensor.matmul(
                                ps[:, mh * 512:(mh + 1) * 512],
                                lhsT=kw_sb[:, kc, dc * 128:(dc + 1) * 128],
                                rhs=encT_sb[:, kc, nh * 1024 + mh * 512:
                                            nh * 1024 + (mh + 1) * 512],
                                start=(kc == 0), stop=(kc == KC - 1))
                    nc.vector.tensor_scalar_add(
                        out=kT_sb[:, dc, nh * 1024:(nh + 1) * 1024], in0=ps,
                        scalar1=kb_sb[:, dc:dc + 1])

            def proj_V_chunk(ncn):
                ps = psum.tile([128, 1024], F32, tag="big", name=f"vps{ncn}")
                for kc in range(KC):
                    nc.tensor.matmul(
                        ps[:, 0:512],
                        lhsT=encT_sb[:, kc, ncn * 128:(ncn + 1) * 128],
                        rhs=vw_sb[:, kc, :],
                        start=(kc == 0), stop=(kc == KC - 1))
                nc.vector.tensor_copy(
                    out=V_sb[:, ncn, :, 0:64],
                    in_=ps[:, 0:512].rearrange("p (h d) -> p h d", h=HL))

            def head_pair(h0, work, etp, outs, emit_mid=None, emit_chunk=None):
                dc = h0 // 2
                pvs = [psum.tile([65, 1024], F32, tag="pv", name=f"pv{h0}_{i}") for i in range(2)]
                etss = [[], []]
                def emit_pv(ncn):
                    for i in range(2):
                        for mh in range(2):
                            nc.tensor.matmul(
                                pvs[i][:, mh * 512:(mh + 1) * 512],
                                lhsT=V_sb[:, ncn, h0 + i, :],
                                rhs=etss[i][ncn][:, mh * 512:(mh + 1) * 512],
                                start=(ncn == 0), stop=(ncn == NCH - 1))

                PV_LAG = 2  # PV consumes exps a couple of chunks old: no stall
                for ncn in range(NCH):
                    if emit_chunk is not None:
                        emit_chunk(ncn)
                    scs = [psum.tile([128, 1024], F32, tag="big", name=f"sc{h0}_{ncn}_{i}") for i in range(2)]
                    # interleave the two heads' K=64 matmuls so the PE runs
                    # them concurrently in different row-groups
                    for mh in range(2):
                        for i, hb in enumerate((0, 64)):
                            nc.tensor.matmul(
                                scs[i][:, mh * 512:(mh + 1) * 512],
                                lhsT=kT_sb[hb:hb + 64, dc,
                                           ncn * 128:(ncn + 1) * 128],
                                rhs=qT_sb[hb:hb + 64, dc,
                                          mh * 512:(mh + 1) * 512],
                                start=True, stop=True)
                    for i in range(2):
                        et = etp.tile([128, 1024], BF16, tag="et")
                        etss[i].append(et)
                        nc.scalar.activation(out=et, in_=scs[i], func=AF.Exp,
                                             bias=mask_sb[:, ncn:ncn + 1],
                                             scale=0.125)
                    if ncn >= PV_LAG:
                        emit_pv(ncn - PV_LAG)
                for ncn in range(NCH - PV_LAG, NCH):
                    emit_pv(ncn)
                def finish(emit_mid2=None):
                    rzs = []
                    for i, hb in enumerate((0, 64)):
                        pv = pvs[i]
                        # 1/Z broadcast to all partitions via K=1 ones matmul
                        zrow = work.tile([1, 1024], F32, tag="zrow", bufs=1,
                                         name=f"zrow{h0}_{i}")
                        nc.scalar.copy(out=zrow, in_=pv[64:65, :])
                        rzrow = work.tile([1, 1024], F32, tag="rzrow", bufs=1,
                                          name=f"rzrow{h0}_{i}")
                        nc.vector.reciprocal_approx_fast(out=rzrow, in_=zrow)
                        rzrow_bf = work.tile([1, 1024], BF16, tag="rzrow_bf",
                                             bufs=1, name=f"rzbf{h0}_{i}")
                        nc.vector.tensor_copy(out=rzrow_bf, in_=rzrow)
                        bc = psum.tile([128, 1024], F32, tag="big",
                                       name=f"bc{h0}_{i}")
                        for mh in range(2):
                            nc.tensor.matmul(bc[:, mh * 512:(mh + 1) * 512],
                                             lhsT=ones_bf,
                                             rhs=rzrow_bf[:, mh * 512:(mh + 1) * 512],
                                             start=True, stop=True)
                        rz = work.tile([128, 1024], BF16, tag="rz",
                                       name=f"rz{h0}_{i}")
                        nc.scalar.activation(out=rz, in_=bc, func=AF.Copy)
                        rzs.append(rz)
                        # critical path: c_proj + the next pair gate on attnT
                        nc.vector.tensor_mul(out=attnT_sb[hb:hb + 64, dc, :],
                                             in0=pv[0:64, :], in1=rz[0:64, :])
                    if emit_mid2 is not None:
                        emit_mid2()
                    for ncn in range(NCH):
                        for i in range(2):
                            pt = outs.tile([128, 1024], BF16, tag="pt",
                                           name=f"pt{h0}_{i}_{ncn}")
                            if ncn in (4, 9, 14):
                                nc.gpsimd.tensor_mul(out=pt, in0=etss[i][ncn],
                                                     in1=rzs[i])
                            else:
                                nc.vector.tensor_mul(out=pt, in0=etss[i][ncn],
                                                     in1=rzs[i])
                            nc.gpsimd.dma_start(
                                out=ptout[h0 + i, ncn * 128:(ncn + 1) * 128, :],
                                in_=pt)
                return finish

            def c_proj(outp, dcs, outs):
                for mc in range(MCH):
                    ps = psum.tile([128, 1024], F32, tag="pv", name=f"cps{mc}")
                    for mh in range(2):
                        for j, dc in enumerate(dcs):
                            nc.tensor.matmul(
                                ps[:, mh * 512:(mh + 1) * 512],
                                lhsT=attnT_sb[:, dc, mc * 128:(mc + 1) * 128],
                                rhs=pw_sb[:, dc, mh * 512:(mh + 1) * 512],
                                start=(j == 0), stop=(j == len(dcs) - 1))
                    ost = outs.tile([128, 1024], F32, tag="ost")
                    nc.vector.tensor_copy(out=ost, in_=ps)
                    nc.sync.dma_start(out=outp[mc * 128:(mc + 1) * 128, :], in_=ost)

            # phase 1: qT for all d-chunks (frees poolB) + kT dc0
            proj_qT(0)
            proj_kT_nh(0, 0)
            proj_kT_nh(0, 1)
            for dc in range(1, DCH):
                proj_qT(dc)
            poolB_cm.__exit__(None, None, None)

            # order matters: work+outs tiles are first touched late (pair
            # tails), so THEY absorb the address-reuse dependency on poolB;
            # the et pool lands above poolB's range and exp can start early
            work = ctx.enter_context(tc.tile_pool(name="work", bufs=2))
            outs = ctx.enter_context(tc.tile_pool(name="outs", bufs=3))
            etp = ctx.enter_context(tc.tile_pool(name="et", bufs=34))

            fins = [None]

            def chunk_work(pair_idx):
                # interleave V (pair 0), the previous pair's tail, and the
                # next pair's kT bursts into this pair's chunk loop
                def emit(ncn):
                    if pair_idx == 0:
                        proj_V_chunk(ncn)
                    elif ncn == 0 and fins[0] is not None:
                        f, fins[0] = fins[0], None
                        f()
                    if pair_idx < DCH - 1:
                        if ncn == 8:
                            proj_kT_nh(pair_idx + 1, 0)
                        elif ncn == 12:
                            proj_kT_nh(pair_idx + 1, 1)
                return emit

            fins[0] = head_pair(0, work, etp, outs, emit_chunk=chunk_work(0))
            f1 = head_pair(2, work, etp, outs, emit_chunk=chunk_work(1))
            fins[0] = lambda: f1(emit_mid2=lambda: c_proj(outp1, (0, 1), outs))
            fins[0] = (lambda prev: prev)(fins[0])
            f2 = head_pair(4, work, etp, outs, emit_chunk=chunk_work(2))
            fins[0] = f2
            f3 = head_pair(6, work, etp, outs, emit_chunk=chunk_work(3))
            f3(emit_mid2=lambda: c_proj(outp2, (2, 3), outs))

    nc.compile()
    return nc


def _get_nc():
    global _NC
    if _NC is None:
        _NC = _build()
    return _NC


def _shard(inputs):
    bf16 = ml_dtypes.bfloat16
    x = np.asarray(inputs["hidden_states"], np.float32)
    enc = np.asarray(inputs["encoder_hidden_states"], np.float32)
    maskf = np.asarray(inputs["attention_mask_kv"], np.float32)
    qw = np.asarray(inputs["q_attn_w"], np.float32)
    qb = np.asarray(inputs["q_attn_b"], np.float32)
    cw = np.asarray(inputs["c_attn_w"], np.float32)
    cb = np.asarray(inputs["c_attn_b"], np.float32)
    pwf = np.asarray(inputs["c_proj_w"], np.float32)

    in_maps = []
    for c in range(8):
        b, g = divmod(c, 2)
        sl = slice(g * DL, (g + 1) * DL)
        vsl = slice(D + g * DL, D + (g + 1) * DL)
        in_maps.append({
            "xT": np.ascontiguousarray(x[b].T).astype(bf16),
            "encT": np.ascontiguousarray(enc[b].T).astype(bf16),
            "qw": np.ascontiguousarray(qw[:, sl]).astype(bf16),
            "kw": np.ascontiguousarray(cw[:, sl]).astype(bf16),
            "vw": np.ascontiguousarray(cw[:, vsl]).astype(bf16),
            "pw": np.ascontiguousarray(pwf[sl, :]).astype(bf16),
            "qb": np.ascontiguousarray(qb[sl].reshape(DCH, 128).T),
            "kb": np.ascontiguousarray(cb[sl].reshape(DCH, 128).T),
            "mask": np.ascontiguousarray(maskf[b, 0, 0].reshape(NCH, 128).T),
        })
    return in_maps


def kernel(**inputs):
    nc = _get_nc()
    in_maps = _shard(inputs)
    res = run_bass_kernel_spmd(nc, in_maps, core_ids=list(range(8)))

    pb = np.asarray(inputs["c_proj_b"], np.float32)
    cb = np.asarray(inputs["c_attn_b"], np.float32)
    pwf = np.asarray(inputs["c_proj_w"], np.float32)
    attn_w = np.empty((B, H, SQ, SKV), np.float32)
    out = np.zeros((B, SQ, D), np.float32)
    for c in range(8):
        b, g = divmod(c, 2)
        pt = res.results[c]["ptout"]           # [8, SKV, SQ]
        attn_w[b, g * HL:(g + 1) * HL] = pt.transpose(0, 2, 1)
        out[b] += res.results[c]["outp1"]
        out[b] += res.results[c]["outp2"]
        out[b] += cb[D + g * DL:D + (g + 1) * DL] @ pwf[g * DL:(g + 1) * DL, :]
    out += pb
    return out, attn_w
